# revision 25
# baseline (speedup 1.0000x reference)
"""Distributed attention kernel for Trainium2 (8 NeuronCores).

Module: x @ w_qkv -> per-head softmax(q k^T / sqrt(hd)) @ v -> out @ w_proj + b.
Shapes: B=2, N=2048, DIM=1024, H=16, HD=64, f32 in/out; bf16 matmul compute
(f32 PSUM accumulation), rel err ~5e-3 vs the f32 reference.

Sharding: core i handles batch b=i//4 and head-group g=i%4 (4 heads).
- qT/kT computed feature-major [256, 2048]; v token-major with a fused
  ones column so the AV matmul also produces softmax denominators.
- Attention per head pair (2j, 2j+1): S^T matmuls go to PE row-groups 0
  and 64 (concurrent on HW), packed side by side in one PSUM tile so a
  single Exp covers both heads; exp on ScalarE straight from PSUM
  (logits are O(1): no max subtraction needed). AV (V stationary,
  producing O^T) is pipelined one strip behind S/exp so ScalarE never
  starves. The j=1 QKV matmuls and the V projection are emitted as PE
  gap-fillers inside pair 0's strips.
- One AllGather per head pair across the 4 cores of a batch group
  (issued as soon as that pair finishes), then each core computes a
  256-column slice of the projection (+bias via a rank-1 K=1 matmul),
  accumulating gather-0 k-tiles first.
Host assembles the 8 per-core [2048, 256] outputs into [2, 2048, 1024].
"""

import sys, os

for _p in ("/opt/trn_rl_repo", "/opt/pypackages"):
    if _p not in sys.path:
        sys.path.insert(0, _p)

import numpy as np
import ml_dtypes
from contextlib import ExitStack

import concourse.bass as bass
import concourse.bacc as bacc
import concourse.mybir as mybir
from concourse import tile
from concourse.bass_utils import run_bass_kernel_spmd

F32 = mybir.dt.float32
BF16 = mybir.dt.bfloat16
NPBF16 = np.dtype(ml_dtypes.bfloat16)

P = 128
NTOK = 2048
C = 1024
NH = 4          # heads per core
HD = 64
FEAT = NH * HD  # 256
KT = C // P     # 8 contraction tiles for qkv
MT = NTOK // P  # 16 token tiles
SCALE = HD ** -0.5
N_CORES = 8
GROUPS = [[0, 1, 2, 3], [4, 5, 6, 7]]

AF = mybir.ActivationFunctionType


def build_program(nc):
    xT = nc.dram_tensor("xT", [C, NTOK], BF16, kind="ExternalInput").ap()
    wq = nc.dram_tensor("wq", [C, FEAT], BF16, kind="ExternalInput").ap()
    wk = nc.dram_tensor("wk", [C, FEAT], BF16, kind="ExternalInput").ap()
    wv = nc.dram_tensor("wv", [C, FEAT], BF16, kind="ExternalInput").ap()
    wp = nc.dram_tensor("wp", [C, FEAT], BF16, kind="ExternalInput").ap()
    bp = nc.dram_tensor("bp", [1, FEAT], BF16, kind="ExternalInput").ap()
    ones_in = nc.dram_tensor("ones", [1, P], BF16, kind="ExternalInput").ap()
    out_e = nc.dram_tensor("out", [NTOK, FEAT], F32, kind="ExternalOutput").ap()

    # Internal DRAM for the collectives (one AllGather per head pair).
    cc_in = [nc.dram_tensor(f"cc_in{j}", [P, NTOK], BF16) for j in range(2)]
    cc_out = [nc.dram_tensor(f"cc_out{j}", [4 * P, NTOK], BF16)
              for j in range(2)]

    with tile.TileContext(nc) as tc, ExitStack() as ctx:
        persist = ctx.enter_context(tc.tile_pool(name="persist", bufs=1))
        psum = ctx.enter_context(tc.tile_pool(name="psum", bufs=1, space="PSUM"))

        # ---- constants ----
        ones_row = persist.tile([1, P], BF16, tag="ones_row")
        nc.sync.dma_start(ones_row[:], ones_in[:])
        ones64 = ones_row[:, 0:64]
        bias_sb = persist.tile([1, FEAT], BF16, tag="bias")
        nc.sync.dma_start(bias_sb[:], bp[:])

        # ---- persistent activations ----
        qT = [persist.tile([P, NTOK], BF16, tag=f"qT{j}", name=f"qT{j}")
              for j in range(2)]
        kT = [persist.tile([P, NTOK], BF16, tag=f"kT{j}", name=f"kT{j}")
              for j in range(2)]
        # v token-major, per head 64 features + a ones column (65 each)
        v_sb = [persist.tile([P, NH * 65], BF16, tag=f"v{m}", name=f"v{m}")
                for m in range(MT)]
        oT = [persist.tile([P, NTOK], BF16, tag=f"oT{j}", name=f"oT{j}")
              for j in range(2)]

        pa = ctx.enter_context(tc.tile_pool(name="stage_a", bufs=1))
        pb = ctx.enter_context(tc.tile_pool(name="stage_b", bufs=1))

        # ---- input loads: weights first (small), x split in halves ----
        x_sb = [pa.tile([P, NTOK], BF16, tag=f"x{k}", name=f"x{k}")
                for k in range(KT)]
        w_sb = {}
        for name, ap in (("wq", wq), ("wk", wk), ("wv", wv)):
            w_sb[name] = [pa.tile([P, FEAT], BF16, tag=f"{name}{k}",
                                  name=f"{name}{k}") for k in range(KT)]
            for k in range(KT):
                nc.sync.dma_start(w_sb[name][k][:], ap[k * P:(k + 1) * P, :])
        for k in range(KT):
            nc.sync.dma_start(x_sb[k][:, 0:NTOK // 2],
                              xT[k * P:(k + 1) * P, 0:NTOK // 2])
            nc.sync.dma_start(x_sb[k][:, NTOK // 2:],
                              xT[k * P:(k + 1) * P, NTOK // 2:])

        # ---- stage A emitters (interleaved into attention as gap fill) ----
        def emit_qk(j, names=("wq", "wk"), chunks=(0, 1, 2, 3)):
            for wname in names:
                dst = qT if wname == "wq" else kT
                for s in chunks:
                    ps = psum.tile([P, 512], F32, tag="ps_a", bufs=2,
                                   name="ps_qk")
                    for k in range(KT):
                        nc.tensor.matmul(
                            ps[:],
                            lhsT=w_sb[wname][k][:, j * P:(j + 1) * P],
                            rhs=x_sb[k][:, s * 512:(s + 1) * 512],
                            start=(k == 0), stop=(k == KT - 1),
                        )
                    nc.vector.tensor_copy(dst[j][:, s * 512:(s + 1) * 512],
                                          ps[:])

        def emit_v(lo=0, hi=MT):
            for m in range(lo, hi):
                ps = psum.tile([P, FEAT], F32, tag="ps_a", bufs=2,
                               padded_shape=[P, 512], name="ps_v")
                for k in range(KT):
                    nc.tensor.matmul(
                        ps[:],
                        lhsT=x_sb[k][:, m * P:(m + 1) * P],
                        rhs=w_sb["wv"][k][:],
                        start=(k == 0), stop=(k == KT - 1),
                    )
                nc.gpsimd.memset(v_sb[m][:], 1.0)
                dst = v_sb[m][:].rearrange("p (h e) -> p h e", e=65)[:, :, 0:64]
                src = ps[:].rearrange("p (h e) -> p h e", e=64)
                nc.vector.tensor_copy(dst, src)

        # ---- stage B: attention, AV pipelined one strip behind S/exp ----
        def emit_av_norm(j, s, pt_tiles):
            m0 = s * 512
            for i in range(2):
                h, po = 2 * j + i, i * 64
                ps_o = psum.tile([65, 512], F32, tag="ps_o", bufs=1)
                for n in range(MT):
                    nc.tensor.matmul(
                        ps_o[:],
                        lhsT=v_sb[n][:, h * 65:(h + 1) * 65],
                        rhs=pt_tiles[n][:, i * 512:(i + 1) * 512],
                        start=(n == 0), stop=(n == MT - 1),
                    )
                # normalize: row 64 of ps_o is the softmax denominator
                rec = pb.tile([1, 512], BF16, tag="rec", bufs=2)
                with nc.allow_low_precision(reason="bf16 recip"):
                    nc.vector.reciprocal(rec[:], ps_o[64:65, :])
                ps_b = psum.tile([64, 512], F32, tag="ps_b", bufs=1)
                nc.tensor.matmul(ps_b[:], lhsT=ones64, rhs=rec[:],
                                 start=True, stop=True)
                bc_sb = pb.tile([64, 512], F32, tag="bc", bufs=2)
                nc.vector.tensor_copy(bc_sb[:], ps_b[:])
                nc.vector.tensor_mul(oT[j][po:po + 64, m0:m0 + 512],
                                     ps_o[0:64, :], bc_sb[:])

        def emit_gather(j):
            half = NTOK // 2
            nc.sync.dma_start(cc_in[j][:, 0:half], oT[j][:, 0:half])
            nc.sync.dma_start(cc_in[j][:, half:], oT[j][:, half:])
            if os.environ.get("KMODE") == "nocc":
                for g in range(4):
                    nc.gpsimd.dma_start(cc_out[j][g * P:(g + 1) * P, :],
                                        cc_in[j][:, :])
            else:
                nc.gpsimd.collective_compute(
                    "AllGather",
                    mybir.AluOpType.bypass,
                    ins=[cc_in[j][:, :]],
                    outs=[cc_out[j][:, :]],
                    replica_groups=GROUPS,
                )

        def emit_s_exp(j, s):
            m0 = s * 512
            pt_tiles = []
            for n in range(MT):
                ps_s = psum.tile([P, 1024], F32, tag="ps_s", bufs=2)
                for i in range(2):      # head 2j at cols 0:512, 2j+1 after
                    po = i * 64
                    nc.tensor.matmul(
                        ps_s[:, i * 512:(i + 1) * 512],
                        lhsT=kT[j][po:po + 64, n * P:(n + 1) * P],
                        rhs=qT[j][po:po + 64, m0:m0 + 512],
                        start=True, stop=True,
                    )
                pt = pb.tile([P, 1024], BF16, tag="pt", bufs=34)
                nc.scalar.activation(pt[:], ps_s[:], AF.Exp, scale=SCALE)
                pt_tiles.append(pt)
            return pt_tiles

        # one flat pipeline over the 8 (pair, strip) steps; AV runs one
        # strip behind S/exp so ScalarE never waits at strip boundaries.
        # Strip 0 needs all of kT0 but only the first qT0 chunk, so emit
        # just those before attention starts; the rest fills PE gaps.
        emit_qk(0, names=("wk",))
        emit_qk(0, names=("wq",), chunks=(0,))
        hooks = {
            0: lambda: (emit_qk(0, names=("wq",), chunks=(1,)), emit_v(0, 8)),
            1: lambda: (emit_qk(0, names=("wq",), chunks=(2,)), emit_v(8, MT)),
            2: lambda: (emit_qk(0, names=("wq",), chunks=(3,)),
                        emit_qk(1, names=("wq",))),
            3: lambda: emit_qk(1, names=("wk",)),
        }
        strips = [(j, s) for j in range(2) for s in range(4)]
        pending = None
        for gi, (j, s) in enumerate(strips):
            if pending is not None and pending[1] == 3:
                # pair boundary: drain the previous pair's last AV before
                # this pair's S so its AllGather launches as early as
                # possible (the gather chain is the serialized tail)
                emit_av_norm(pending[0], pending[1], pending[2])
                emit_gather(pending[0])
                pending = None
            pt_tiles = emit_s_exp(j, s)
            if gi in hooks:
                hooks[gi]()             # PE gap fill under the exp stream
            if pending is not None:
                emit_av_norm(pending[0], pending[1], pending[2])
            pending = (j, s, pt_tiles)
        emit_av_norm(pending[0], pending[1], pending[2])
        emit_gather(pending[0])

        # scheduler-only fence: keep stage D's PE work out of the attention
        # stream (PE is strict FIFO; an early proj ldweights waiting on the
        # gather would block everything behind it)
        tc.no_sync_barrier()

        # ---- stage D: projection column slice, two passes ----
        # pass 1 (after gather 0): accumulate even k-tiles into SBUF;
        # pass 2 (after gather 1): odd k-tiles + bias, add pass-1 partials.
        with tc.tile_pool(name="stage_d", bufs=1) as pd:
            # reuse the x slots (same shape/dtype, long dead by now)
            ot_full = [pa.tile([P, NTOK], BF16, tag=f"x{k}", name=f"of{k}")
                       for k in range(KT)]
            wp_sb = [pd.tile([P, FEAT], BF16, tag=f"wp{k}", name=f"wp{k}")
                     for k in range(KT)]
            for k in range(KT):
                nc.sync.dma_start(wp_sb[k][:], wp[k * P:(k + 1) * P, :])
            K_ORDER = [0, 2, 4, 6, 1, 3, 5, 7]  # pair-0 gather lands first
            for k in K_ORDER:       # halves spread the queue load
                half = NTOK // 2
                src_ap = cc_out[k % 2][(k // 2) * P:(k // 2 + 1) * P, :]
                nc.sync.dma_start(ot_full[k][:, 0:half], src_ap[:, 0:half])
                nc.sync.dma_start(ot_full[k][:, half:], src_ap[:, half:])

            acc = [pd.tile([P, FEAT], F32, tag=f"acc{m}", name=f"acc{m}")
                   for m in range(MT)]
            for m in range(MT):
                ps = psum.tile([P, FEAT], F32, tag="ps_a", bufs=2,
                               padded_shape=[P, 512], name="ps_proj1")
                for ki, k in enumerate(K_ORDER[:4]):
                    nc.tensor.matmul(
                        ps[:],
                        lhsT=ot_full[k][:, m * P:(m + 1) * P],
                        rhs=wp_sb[k][:],
                        start=(ki == 0), stop=(ki == 3),
                    )
                nc.vector.tensor_copy(acc[m][:], ps[:])
            for m in range(MT):
                ps = psum.tile([P, FEAT], F32, tag="ps_a", bufs=2,
                               padded_shape=[P, 512], name="ps_proj2")
                for ki, k in enumerate(K_ORDER[4:]):
                    nc.tensor.matmul(
                        ps[:],
                        lhsT=ot_full[k][:, m * P:(m + 1) * P],
                        rhs=wp_sb[k][:],
                        start=(ki == 0), stop=False,
                    )
                nc.tensor.matmul(ps[:], lhsT=ones_row[:], rhs=bias_sb[:],
                                 start=False, stop=True)
                o_sb = pd.tile([P, FEAT], F32, tag="osb", bufs=3)
                nc.vector.tensor_add(o_sb[:], ps[:], acc[m][:])
                nc.sync.dma_start(out_e[m * P:(m + 1) * P, :], o_sb[:])

    return nc


_CACHE = {}


def _get_nc():
    if "nc" not in _CACHE:
        nc = bacc.Bacc("TRN2", target_bir_lowering=False, debug=False,
                       num_devices=N_CORES)
        nc = build_program(nc)
        nc.compile()
        _CACHE["nc"] = nc
    return _CACHE["nc"]


def make_in_maps(x, w_qkv, w_proj, b_proj):
    in_maps = []
    for core in range(N_CORES):
        b, g = core // 4, core % 4
        hs = slice(g * FEAT, (g + 1) * FEAT)
        in_maps.append({
            "xT": np.ascontiguousarray(x[b].T).astype(NPBF16),
            "wq": np.ascontiguousarray(w_qkv[:, 0:1024][:, hs]).astype(NPBF16),
            "wk": np.ascontiguousarray(w_qkv[:, 1024:2048][:, hs]).astype(NPBF16),
            "wv": np.ascontiguousarray(w_qkv[:, 2048:3072][:, hs]).astype(NPBF16),
            "wp": np.ascontiguousarray(w_proj[:, hs]).astype(NPBF16),
            "bp": np.ascontiguousarray(b_proj[hs]).reshape(1, FEAT).astype(NPBF16),
            "ones": np.ones((1, P), NPBF16),
        })
    return in_maps


def assemble(results):
    out = np.empty((2, NTOK, 1024), np.float32)
    for core in range(N_CORES):
        b, g = core // 4, core % 4
        out[b][:, g * FEAT:(g + 1) * FEAT] = results[core]["out"]
    return out


def kernel(x, w_qkv, w_proj, b_proj, trace=False):
    nc = _get_nc()
    in_maps = make_in_maps(np.asarray(x), np.asarray(w_qkv),
                           np.asarray(w_proj), np.asarray(b_proj))
    res = run_bass_kernel_spmd(nc, in_maps, core_ids=list(range(N_CORES)),
                               trace=trace)
    out = assemble(res.results)
    if trace:
        return out, res
    return out


# revision 26
# speedup vs baseline: 1.0130x; 1.0130x over previous
"""Distributed attention kernel for Trainium2 (8 NeuronCores).

Module: x @ w_qkv -> per-head softmax(q k^T / sqrt(hd)) @ v -> out @ w_proj + b.
Shapes: B=2, N=2048, DIM=1024, H=16, HD=64, f32 in/out; bf16 matmul compute
(f32 PSUM accumulation), rel err ~5e-3 vs the f32 reference.

Sharding: core i handles batch b=i//4 and head-group g=i%4 (4 heads).
- qT/kT computed feature-major [256, 2048]; v token-major with a fused
  ones column so the AV matmul also produces softmax denominators.
- Attention per head pair (2j, 2j+1): S^T matmuls go to PE row-groups 0
  and 64 (concurrent on HW), packed side by side in one PSUM tile so a
  single Exp covers both heads; exp on ScalarE straight from PSUM
  (logits are O(1): no max subtraction needed). AV (V stationary,
  producing O^T) is pipelined one strip behind S/exp so ScalarE never
  starves. The j=1 QKV matmuls and the V projection are emitted as PE
  gap-fillers inside pair 0's strips.
- One AllGather per head pair across the 4 cores of a batch group
  (issued as soon as that pair finishes), then each core computes a
  256-column slice of the projection (+bias via a rank-1 K=1 matmul),
  accumulating gather-0 k-tiles first.
Host assembles the 8 per-core [2048, 256] outputs into [2, 2048, 1024].
"""

import sys, os

for _p in ("/opt/trn_rl_repo", "/opt/pypackages"):
    if _p not in sys.path:
        sys.path.insert(0, _p)

import numpy as np
import ml_dtypes
from contextlib import ExitStack

import concourse.bass as bass
import concourse.bacc as bacc
import concourse.mybir as mybir
from concourse import tile
from concourse.bass_utils import run_bass_kernel_spmd

F32 = mybir.dt.float32
BF16 = mybir.dt.bfloat16
NPBF16 = np.dtype(ml_dtypes.bfloat16)

P = 128
NTOK = 2048
C = 1024
NH = 4          # heads per core
HD = 64
FEAT = NH * HD  # 256
KT = C // P     # 8 contraction tiles for qkv
MT = NTOK // P  # 16 token tiles
SCALE = HD ** -0.5
N_CORES = 8
GROUPS = [[0, 1, 2, 3], [4, 5, 6, 7]]

AF = mybir.ActivationFunctionType


def build_program(nc):
    xT = nc.dram_tensor("xT", [C, NTOK], BF16, kind="ExternalInput").ap()
    wq = nc.dram_tensor("wq", [C, FEAT], BF16, kind="ExternalInput").ap()
    wk = nc.dram_tensor("wk", [C, FEAT], BF16, kind="ExternalInput").ap()
    wv = nc.dram_tensor("wv", [C, FEAT], BF16, kind="ExternalInput").ap()
    wp = nc.dram_tensor("wp", [C, FEAT], BF16, kind="ExternalInput").ap()
    bp = nc.dram_tensor("bp", [1, FEAT], BF16, kind="ExternalInput").ap()
    ones_in = nc.dram_tensor("ones", [1, P], BF16, kind="ExternalInput").ap()
    out_e = nc.dram_tensor("out", [NTOK, FEAT], F32, kind="ExternalOutput").ap()

    # Internal DRAM for the collectives (one AllGather per head pair).
    cc_in = [nc.dram_tensor(f"cc_in{j}", [P, NTOK], BF16) for j in range(2)]
    cc_out = [nc.dram_tensor(f"cc_out{j}", [4 * P, NTOK], BF16)
              for j in range(2)]

    with tile.TileContext(nc) as tc, ExitStack() as ctx:
        persist = ctx.enter_context(tc.tile_pool(name="persist", bufs=1))
        psum = ctx.enter_context(tc.tile_pool(name="psum", bufs=1, space="PSUM"))

        # ---- constants ----
        ones_row = persist.tile([1, P], BF16, tag="ones_row")
        nc.sync.dma_start(ones_row[:], ones_in[:])
        ones64 = ones_row[:, 0:64]
        bias_sb = persist.tile([1, FEAT], BF16, tag="bias")
        nc.sync.dma_start(bias_sb[:], bp[:])

        # ---- persistent activations ----
        qT = [persist.tile([P, NTOK], BF16, tag=f"qT{j}", name=f"qT{j}")
              for j in range(2)]
        kT = [persist.tile([P, NTOK], BF16, tag=f"kT{j}", name=f"kT{j}")
              for j in range(2)]
        # v token-major, per head 64 features + a ones column (65 each)
        v_sb = [persist.tile([P, NH * 65], BF16, tag=f"v{m}", name=f"v{m}")
                for m in range(MT)]
        oT = [persist.tile([P, NTOK], BF16, tag=f"oT{j}", name=f"oT{j}")
              for j in range(2)]

        pa = ctx.enter_context(tc.tile_pool(name="stage_a", bufs=1))
        pb = ctx.enter_context(tc.tile_pool(name="stage_b", bufs=1))

        # ---- input loads: weights first (small), x split in halves ----
        x_sb = [pa.tile([P, NTOK], BF16, tag=f"x{k}", name=f"x{k}")
                for k in range(KT)]
        w_sb = {}
        for name, ap in (("wq", wq), ("wk", wk), ("wv", wv)):
            w_sb[name] = [pa.tile([P, FEAT], BF16, tag=f"{name}{k}",
                                  name=f"{name}{k}") for k in range(KT)]
            for k in range(KT):
                nc.sync.dma_start(w_sb[name][k][:], ap[k * P:(k + 1) * P, :])
        for k in range(KT):
            nc.sync.dma_start(x_sb[k][:, 0:NTOK // 2],
                              xT[k * P:(k + 1) * P, 0:NTOK // 2])
            nc.sync.dma_start(x_sb[k][:, NTOK // 2:],
                              xT[k * P:(k + 1) * P, NTOK // 2:])

        # ---- stage A emitters (interleaved into attention as gap fill) ----
        def emit_qk(j, names=("wq", "wk"), chunks=(0, 1, 2, 3)):
            for wname in names:
                dst = qT if wname == "wq" else kT
                for s in chunks:
                    ps = psum.tile([P, 512], F32, tag="ps_a", bufs=2,
                                   name="ps_qk")
                    for k in range(KT):
                        nc.tensor.matmul(
                            ps[:],
                            lhsT=w_sb[wname][k][:, j * P:(j + 1) * P],
                            rhs=x_sb[k][:, s * 512:(s + 1) * 512],
                            start=(k == 0), stop=(k == KT - 1),
                        )
                    nc.vector.tensor_copy(dst[j][:, s * 512:(s + 1) * 512],
                                          ps[:])

        def emit_v(lo=0, hi=MT):
            for m in range(lo, hi):
                ps = psum.tile([P, FEAT], F32, tag="ps_a", bufs=2,
                               padded_shape=[P, 512], name="ps_v")
                for k in range(KT):
                    nc.tensor.matmul(
                        ps[:],
                        lhsT=x_sb[k][:, m * P:(m + 1) * P],
                        rhs=w_sb["wv"][k][:],
                        start=(k == 0), stop=(k == KT - 1),
                    )
                nc.gpsimd.memset(v_sb[m][:], 1.0)
                dst = v_sb[m][:].rearrange("p (h e) -> p h e", e=65)[:, :, 0:64]
                src = ps[:].rearrange("p (h e) -> p h e", e=64)
                nc.vector.tensor_copy(dst, src)

        # ---- stage B: attention, AV pipelined one strip behind S/exp ----
        def emit_av_norm(j, s, pt_tiles):
            m0 = s * 512
            for i in range(2):
                h, po = 2 * j + i, i * 64
                ps_o = psum.tile([65, 512], F32, tag="ps_o", bufs=1)
                for n in range(MT):
                    nc.tensor.matmul(
                        ps_o[:],
                        lhsT=v_sb[n][:, h * 65:(h + 1) * 65],
                        rhs=pt_tiles[n][:, i * 512:(i + 1) * 512],
                        start=(n == 0), stop=(n == MT - 1),
                    )
                # normalize: row 64 of ps_o is the softmax denominator
                rec = pb.tile([1, 512], BF16, tag="rec", bufs=2)
                with nc.allow_low_precision(reason="bf16 recip"):
                    nc.vector.reciprocal(rec[:], ps_o[64:65, :])
                ps_b = psum.tile([64, 512], F32, tag="ps_b", bufs=1)
                nc.tensor.matmul(ps_b[:], lhsT=ones64, rhs=rec[:],
                                 start=True, stop=True)
                bc_sb = pb.tile([64, 512], F32, tag="bc", bufs=2)
                nc.vector.tensor_copy(bc_sb[:], ps_b[:])
                nc.vector.tensor_mul(oT[j][po:po + 64, m0:m0 + 512],
                                     ps_o[0:64, :], bc_sb[:])

        def emit_gather(j):
            half = NTOK // 2
            nc.sync.dma_start(cc_in[j][:, 0:half], oT[j][:, 0:half])
            nc.sync.dma_start(cc_in[j][:, half:], oT[j][:, half:])
            if os.environ.get("KMODE") == "nocc":
                for g in range(4):
                    nc.gpsimd.dma_start(cc_out[j][g * P:(g + 1) * P, :],
                                        cc_in[j][:, :])
            else:
                nc.gpsimd.collective_compute(
                    "AllGather",
                    mybir.AluOpType.bypass,
                    ins=[cc_in[j][:, :]],
                    outs=[cc_out[j][:, :]],
                    replica_groups=GROUPS,
                )

        def emit_s_exp(j, s):
            m0 = s * 512
            pt_tiles = []
            for n in range(MT):
                ps_s = psum.tile([P, 1024], F32, tag="ps_s", bufs=2)
                for i in range(2):      # head 2j at cols 0:512, 2j+1 after
                    po = i * 64
                    nc.tensor.matmul(
                        ps_s[:, i * 512:(i + 1) * 512],
                        lhsT=kT[j][po:po + 64, n * P:(n + 1) * P],
                        rhs=qT[j][po:po + 64, m0:m0 + 512],
                        start=True, stop=True,
                    )
                pt = pb.tile([P, 1024], BF16, tag="pt", bufs=34)
                nc.scalar.activation(pt[:], ps_s[:], AF.Exp, scale=SCALE)
                pt_tiles.append(pt)
            return pt_tiles

        # one flat pipeline over the 8 (pair, strip) steps; AV runs one
        # strip behind S/exp so ScalarE never waits at strip boundaries.
        # Strip 0 needs all of kT0 but only the first qT0 chunk, so emit
        # just those before attention starts; the rest fills PE gaps.
        emit_qk(0, names=("wk",))
        emit_qk(0, names=("wq",), chunks=(0,))
        # gap-fill balanced against the exp pace (~18.4us per strip):
        # v must be fully emitted before the first AV (PE is strict FIFO,
        # a later v matmul would deadlock an earlier AV that reads it);
        # qT1 chunk s is only needed by pair-1 strip s, so qk(1) spreads
        # deep into pair 1's hook slots.
        hooks = {
            0: lambda: (emit_qk(0, names=("wq",), chunks=(1,)), emit_v(0, 11)),
            1: lambda: (emit_qk(0, names=("wq",), chunks=(2,)), emit_v(11, MT)),
            2: lambda: (emit_qk(0, names=("wq",), chunks=(3,)),
                        emit_qk(1, names=("wk",), chunks=(0, 1))),
            3: lambda: (emit_qk(1, names=("wk",), chunks=(2, 3)),
                        emit_qk(1, names=("wq",), chunks=(0,))),
            4: lambda: emit_qk(1, names=("wq",), chunks=(1,)),
            5: lambda: emit_qk(1, names=("wq",), chunks=(2,)),
            6: lambda: emit_qk(1, names=("wq",), chunks=(3,)),
        }
        strips = [(j, s) for j in range(2) for s in range(4)]
        pending = None
        for gi, (j, s) in enumerate(strips):
            if pending is not None and pending[1] == 3:
                # pair boundary: drain the previous pair's last AV before
                # this pair's S so its AllGather launches as early as
                # possible (the gather chain is the serialized tail)
                emit_av_norm(pending[0], pending[1], pending[2])
                emit_gather(pending[0])
                pending = None
            pt_tiles = emit_s_exp(j, s)
            if gi in hooks:
                hooks[gi]()             # PE gap fill under the exp stream
            if pending is not None:
                emit_av_norm(pending[0], pending[1], pending[2])
            pending = (j, s, pt_tiles)
        emit_av_norm(pending[0], pending[1], pending[2])
        emit_gather(pending[0])

        # scheduler-only fence: keep stage D's PE work out of the attention
        # stream (PE is strict FIFO; an early proj ldweights waiting on the
        # gather would block everything behind it)
        tc.no_sync_barrier()

        # ---- stage D: projection column slice, two passes ----
        # pass 1 (after gather 0): accumulate even k-tiles into SBUF;
        # pass 2 (after gather 1): odd k-tiles + bias, add pass-1 partials.
        with tc.tile_pool(name="stage_d", bufs=1) as pd:
            # reuse the x slots (same shape/dtype, long dead by now)
            ot_full = [pa.tile([P, NTOK], BF16, tag=f"x{k}", name=f"of{k}")
                       for k in range(KT)]
            wp_sb = [pd.tile([P, FEAT], BF16, tag=f"wp{k}", name=f"wp{k}")
                     for k in range(KT)]
            for k in range(KT):
                nc.sync.dma_start(wp_sb[k][:], wp[k * P:(k + 1) * P, :])
            K_ORDER = [0, 2, 4, 6, 1, 3, 5, 7]  # pair-0 gather lands first
            for k in K_ORDER:       # halves spread the queue load
                half = NTOK // 2
                src_ap = cc_out[k % 2][(k // 2) * P:(k // 2 + 1) * P, :]
                nc.sync.dma_start(ot_full[k][:, 0:half], src_ap[:, 0:half])
                nc.sync.dma_start(ot_full[k][:, half:], src_ap[:, half:])

            acc = [pd.tile([P, FEAT], F32, tag=f"acc{m}", name=f"acc{m}")
                   for m in range(MT)]
            for m in range(MT):
                ps = psum.tile([P, FEAT], F32, tag="ps_a", bufs=2,
                               padded_shape=[P, 512], name="ps_proj1")
                for ki, k in enumerate(K_ORDER[:4]):
                    nc.tensor.matmul(
                        ps[:],
                        lhsT=ot_full[k][:, m * P:(m + 1) * P],
                        rhs=wp_sb[k][:],
                        start=(ki == 0), stop=(ki == 3),
                    )
                nc.vector.tensor_copy(acc[m][:], ps[:])
            for m in range(MT):
                ps = psum.tile([P, FEAT], F32, tag="ps_a", bufs=2,
                               padded_shape=[P, 512], name="ps_proj2")
                for ki, k in enumerate(K_ORDER[4:]):
                    nc.tensor.matmul(
                        ps[:],
                        lhsT=ot_full[k][:, m * P:(m + 1) * P],
                        rhs=wp_sb[k][:],
                        start=(ki == 0), stop=False,
                    )
                nc.tensor.matmul(ps[:], lhsT=ones_row[:], rhs=bias_sb[:],
                                 start=False, stop=True)
                o_sb = pd.tile([P, FEAT], F32, tag="osb", bufs=3)
                nc.vector.tensor_add(o_sb[:], ps[:], acc[m][:])
                nc.sync.dma_start(out_e[m * P:(m + 1) * P, :], o_sb[:])

    return nc


_CACHE = {}


def _get_nc():
    if "nc" not in _CACHE:
        nc = bacc.Bacc("TRN2", target_bir_lowering=False, debug=False,
                       num_devices=N_CORES)
        nc = build_program(nc)
        nc.compile()
        _CACHE["nc"] = nc
    return _CACHE["nc"]


def make_in_maps(x, w_qkv, w_proj, b_proj):
    in_maps = []
    for core in range(N_CORES):
        b, g = core // 4, core % 4
        hs = slice(g * FEAT, (g + 1) * FEAT)
        in_maps.append({
            "xT": np.ascontiguousarray(x[b].T).astype(NPBF16),
            "wq": np.ascontiguousarray(w_qkv[:, 0:1024][:, hs]).astype(NPBF16),
            "wk": np.ascontiguousarray(w_qkv[:, 1024:2048][:, hs]).astype(NPBF16),
            "wv": np.ascontiguousarray(w_qkv[:, 2048:3072][:, hs]).astype(NPBF16),
            "wp": np.ascontiguousarray(w_proj[:, hs]).astype(NPBF16),
            "bp": np.ascontiguousarray(b_proj[hs]).reshape(1, FEAT).astype(NPBF16),
            "ones": np.ones((1, P), NPBF16),
        })
    return in_maps


def assemble(results):
    out = np.empty((2, NTOK, 1024), np.float32)
    for core in range(N_CORES):
        b, g = core // 4, core % 4
        out[b][:, g * FEAT:(g + 1) * FEAT] = results[core]["out"]
    return out


def kernel(x, w_qkv, w_proj, b_proj, trace=False):
    nc = _get_nc()
    in_maps = make_in_maps(np.asarray(x), np.asarray(w_qkv),
                           np.asarray(w_proj), np.asarray(b_proj))
    res = run_bass_kernel_spmd(nc, in_maps, core_ids=list(range(N_CORES)),
                               trace=trace)
    out = assemble(res.results)
    if trace:
        return out, res
    return out
